# revision 10
# baseline (speedup 1.0000x reference)
"""Grouped self-attention (B=2, S=2048, D=1024, H=16, hd=64) on 8 trn2 cores.

Sharding: core c = b*4 + g handles batch b, heads [4g, 4g+4).

Key simplification: the reference's RoPE indexes its cos/sin cache by the
BATCH dim and uses neg_half = [t_first, -t_second], so rope(t)[b,s,h,d] =
t * (cos(b*th[d%32]) + sign(d)*sin(b*th[d%32])) — a pure per-(b,d) scale
that folds into rows of Wq/Wk on the host. The device kernel is then just
QKV projection + softmax attention.

v2: all matmul operands bf16 (fp32r streams at half clock and draws PE
throttling; PSUM accumulation stays f32). PV is computed as
out[q,d] = et_chunk.T @ V_aug directly (et chunks as stationary operand),
which kills the PE transposes + PSUM->SBUF staging of v1; softmax
denominator comes from an appended ones-column in V_aug. ACT engine does
exp only; all PSUM->SBUF copies are split between DVE and GpSimd.

Device layout per core:
  xt    [128, 8*2048] bf16 = x[b].T        (8 k-major chunks)
  wqt/wkt/wvt [128, 8*256] bf16            (rope folded into wq/wk rows)
  qt/kt [2][128, 2048] bf16                (2 head-pairs x 64d rows)
  v_sb  [128, 16, 4, 65] bf16              (s-chunk part, 4 heads, d+ones)
  maskb [128, 16] f32                      ((mask-1)*3e4 bias per kb chunk)
  ost   [128, 16, 256] f32 -> out [2048, 256]
"""

import numpy as np
from contextlib import ExitStack

import ml_dtypes
import concourse.bass as bass
import concourse.bacc as bacc
import concourse.tile as tile
from concourse import mybir
from concourse.bass_utils import run_bass_kernel_spmd

F32 = mybir.dt.float32
BF16 = mybir.dt.bfloat16
EXP = mybir.ActivationFunctionType.Exp

B, S, D, H, HD = 2, 2048, 1024, 16, 64
NCORES = 8

_CACHE = {}


def _build_nc():
    nc = bacc.Bacc("TRN2", target_bir_lowering=False, debug=False)
    xt_d = nc.declare_dram_parameter("xt", [8, 128, S], BF16, isOutput=False)
    wqt_d = nc.declare_dram_parameter("wqt", [8, 128, 256], BF16, isOutput=False)
    wkt_d = nc.declare_dram_parameter("wkt", [8, 128, 256], BF16, isOutput=False)
    wvt_d = nc.declare_dram_parameter("wvt", [8, 128, 256], BF16, isOutput=False)
    mb_d = nc.declare_dram_parameter("maskb", [128, 16], F32, isOutput=False)
    out_d = nc.declare_dram_parameter("out", [S, 256], F32, isOutput=True)

    with tile.TileContext(nc) as tc, ExitStack() as ctx:
        const = ctx.enter_context(tc.tile_pool(name="const", bufs=1))
        xpool = ctx.enter_context(tc.tile_pool(name="x", bufs=1))
        wpool = ctx.enter_context(tc.tile_pool(name="w", bufs=1))
        qkpool = ctx.enter_context(tc.tile_pool(name="qk", bufs=1))
        vpool = ctx.enter_context(tc.tile_pool(name="v", bufs=1))
        opool = ctx.enter_context(tc.tile_pool(name="o", bufs=1))
        epool = ctx.enter_context(tc.tile_pool(name="et", bufs=3))
        small = ctx.enter_context(tc.tile_pool(name="small", bufs=4))
        scp = ctx.enter_context(tc.tile_pool(name="scp", bufs=2, space="PSUM"))
        pvp = ctx.enter_context(tc.tile_pool(name="pvp", bufs=2, space="PSUM"))

        mb = const.tile([128, 16], F32)
        nc.sync.dma_start(mb[:], mb_d[:])

        xt = xpool.tile([128, 8 * S], BF16)
        for c in range(8):
            nc.sync.dma_start(xt[:, c * S:(c + 1) * S], xt_d[c])
        wq = wpool.tile([128, 8 * 256], BF16, tag="wq")
        wk = wpool.tile([128, 8 * 256], BF16, tag="wk")
        wv = wpool.tile([128, 8 * 256], BF16, tag="wv")
        for wtile, wd in ((wq, wqt_d), (wk, wkt_d), (wv, wvt_d)):
            for c in range(8):
                nc.sync.dma_start(wtile[:, c * 256:(c + 1) * 256], wd[c])

        # ---- phase 1: projections ----
        # QT/KT per 2-head pair: [128 (2h x 64d), 2048 s]
        qt = [qkpool.tile([128, S], BF16, tag=f"qt{p}", name=f"qt{p}") for p in range(2)]
        kt = [qkpool.tile([128, S], BF16, tag=f"kt{p}", name=f"kt{p}") for p in range(2)]
        for pair in range(2):
            for wtile, dst in ((wq, qt[pair]), (wk, kt[pair])):
                for nb in range(4):
                    ps = scp.tile([128, 1024], F32, tag="sc")
                    for kc in range(8):
                        lo = kc * 256 + pair * 128
                        nc.tensor.matmul(
                            ps[:, 0:512],
                            lhsT=wtile[:, lo:lo + 128],
                            rhs=xt[:, kc * S + nb * 512: kc * S + nb * 512 + 512],
                            start=(kc == 0), stop=(kc == 7))
                    nc.vector.tensor_copy(dst[:, nb * 512:(nb + 1) * 512],
                                          ps[:, 0:512])

        # V (all 4 heads): [s part chunks, 4h x 65] with ones col
        v_sb = vpool.tile([128, 16, 4, 65], BF16)
        nc.vector.memset(v_sb[:, :, :, 64:65], 1.0)
        for m in range(16):
            pv = scp.tile([128, 1024], F32, tag="sc")
            for kc in range(8):
                nc.tensor.matmul(
                    pv[:, 0:256],
                    lhsT=xt[:, kc * S + m * 128: kc * S + m * 128 + 128],
                    rhs=wv[:, kc * 256:(kc + 1) * 256],
                    start=(kc == 0), stop=(kc == 7))
            nc.vector.tensor_copy(v_sb[:, m, :, 0:64], pv[:, 0:256])

        # ---- phase 2: attention ----
        # scores [k part, q free] -> exp -> PV: out[q,d] = et_chunk.T @ V_aug
        ost = opool.tile([128, 16, 256], F32)
        for qh in range(2):
            for h in range(4):
                pair, hoff = divmod(h, 2)
                hoff *= 64
                # 4 accumulation slices share a PSUM bank and a matmul with
                # start=True zeroes the WHOLE bank, so pre-zero via DVE and
                # accumulate with start=False on every PV matmul.
                pva = pvp.tile([128, 4, 65], F32, tag="pva")
                pvb = pvp.tile([128, 4, 65], F32, tag="pvb")
                nc.vector.memset(pva[:], 0.0)
                nc.vector.memset(pvb[:], 0.0)
                for kb in range(16):
                    ps = scp.tile([128, 1024], F32, tag="sc")
                    for j in range(2):
                        q0 = qh * 1024 + j * 512
                        nc.tensor.matmul(
                            ps[:, j * 512:(j + 1) * 512],
                            lhsT=kt[pair][hoff:hoff + 64, kb * 128:(kb + 1) * 128],
                            rhs=qt[pair][hoff:hoff + 64, q0:q0 + 512],
                            start=True, stop=True)
                    et = epool.tile([128, 1024], BF16)
                    nc.scalar.activation(et[:], ps[:], EXP,
                                         bias=mb[:, kb:kb + 1], scale=0.125)
                    for t in range(8):
                        dst = pva if t < 4 else pvb
                        nc.tensor.matmul(
                            dst[:, t % 4, :],
                            lhsT=et[:, t * 128:(t + 1) * 128],
                            rhs=v_sb[:, kb, h, :],
                            start=False, stop=(kb == 15),
                            skip_group_check=True)
                rca = small.tile([128, 4, 1], F32, tag="rca")
                rcb = small.tile([128, 4, 1], F32, tag="rcb")
                nc.vector.reciprocal(rca[:], pva[:, :, 64:65])
                nc.vector.reciprocal(rcb[:], pvb[:, :, 64:65])
                for t in range(8):
                    src = pva if t < 4 else pvb
                    rc = rca if t < 4 else rcb
                    nc.vector.tensor_scalar_mul(
                        ost[:, qh * 8 + t, hoff + (pair * 128):hoff + (pair * 128) + 64],
                        src[:, t % 4, 0:64], rc[:, t % 4, :])
            for m in range(qh * 8, qh * 8 + 8):
                nc.sync.dma_start(out_d[m * 128:(m + 1) * 128, :], ost[:, m, :])
    nc.compile()
    return nc


def _host_prep(x, attention_mask, Wq, Wk, Wv):
    x = np.asarray(x, dtype=np.float32)
    mask = np.asarray(attention_mask)
    Wq = np.asarray(Wq, dtype=np.float32)
    Wk = np.asarray(Wk, dtype=np.float32)
    Wv = np.asarray(Wv, dtype=np.float32)
    bf16 = ml_dtypes.bfloat16

    # rope fold: c_eff[b, d] = cos(b*th[d%32]) + sign(d)*sin(b*th[d%32])
    j = np.arange(0, HD, 2, dtype=np.float64) / HD          # [32]
    theta = 1.0 / (10000.0 ** j)                            # [32]
    dd = np.arange(HD)
    sign = np.where(dd < 32, 1.0, -1.0)
    in_maps = []
    wvt_full = np.ascontiguousarray(Wv.T).astype(bf16)      # [1024,1024]
    for b in range(B):
        ang = b * theta                                     # [32]
        ce = np.cos(ang[dd % 32]) + sign * np.sin(ang[dd % 32])  # [64]
        ccol = np.tile(ce, H).astype(np.float32)            # [1024]
        wqt_full = np.ascontiguousarray((Wq * ccol[:, None]).T).astype(bf16)
        wkt_full = np.ascontiguousarray((Wk * ccol[:, None]).T).astype(bf16)
        xt = np.ascontiguousarray(x[b].T).astype(bf16).reshape(8, 128, S)
        maskb = np.ascontiguousarray(
            ((mask[b].astype(np.float32) - 1.0) * 30000.0).reshape(16, 128).T)
        for g in range(4):
            cols = slice(g * 256, (g + 1) * 256)
            in_maps.append({
                "xt": xt,
                "wqt": np.ascontiguousarray(wqt_full[:, cols]).reshape(8, 128, 256),
                "wkt": np.ascontiguousarray(wkt_full[:, cols]).reshape(8, 128, 256),
                "wvt": np.ascontiguousarray(wvt_full[:, cols]).reshape(8, 128, 256),
                "maskb": maskb,
            })
    return in_maps


def _get_nc():
    if "nc" not in _CACHE:
        _CACHE["nc"] = _build_nc()
    return _CACHE["nc"]


def kernel(x, attention_mask, Wq, Wk, Wv, **extra_kwargs):
    nc = _get_nc()
    in_maps = _host_prep(x, attention_mask, Wq, Wk, Wv)
    res = run_bass_kernel_spmd(nc, in_maps, list(range(NCORES))).results
    out = np.empty((B, S, D), dtype=np.float32)
    for c in range(NCORES):
        b, g = divmod(c, 4)
        out[b, :, g * 256:(g + 1) * 256] = res[c]["out"]
    return out
